# revision 6
# baseline (speedup 1.0000x reference)
"""Trainium2 Bass kernel for LocalRNN (sliding-window GRU, B=16 L=1024 D=256 K=16).

Strategy (8 NeuronCores, data-parallel over batch, 2 batch rows / core):
  - For each of the 2048 positions per core, a 16-step GRU runs over the
    trailing 16-position window (zero left-pad).  All math is done in a
    feature-major layout: features on SBUF partitions (256 = 2x128 halves),
    window index on the free axis.
  - gi = W_ih @ x_t is precomputed once per *position* (16x reuse across the
    overlapping windows) and stored fp16 in SBUF with a 15-column pad head.
  - The GRU is rewritten in sigmoid space:  h = 2*ht - 1, so that
        ht' = u + z*(ht - u),  u = sigmoid(2*a_n)
    and tanh disappears (tanh(v) = 2*sigmoid(2v) - 1).  All gate scales and
    biases are folded into the (host-prepared) weights:
        a_rz = gi_rz + (2*W_hh_rz) @ ht
        B    = (4*W_hh_n) @ ht + 2*(b_hh_n - W_hh_n@1)
        a_n  = gi_n + r * B
    gi_rz carries b_ih_rz + b_hh_rz - W_hh_rz@1 and gi_n carries 2*b_ih_n.
  - Matmuls accumulate in fp32 PSUM; gi adds for r/z ride the same PSUM
    accumulation group via identity matmuls, so the sigmoid reads PSUM
    directly.  Elementwise runs fp16 on the vector engine (2x mode).
  - Output ht is xbar-DMA-transposed back to row-major fp16; the final
    affine 2*ht-1 and fp32 cast happen on the host.
"""

import numpy as np

import concourse.bacc as bacc
import concourse.bass as bass
import concourse.mybir as mybir
from concourse import bass_utils
from concourse.tile import TileContext

F16 = mybir.dt.float16
F32 = mybir.dt.float32
AF = mybir.ActivationFunctionType
ALU = mybir.AluOpType

B, L, D, KS = 16, 1024, 256, 16
NCORES = 8
BPC = B // NCORES          # batch rows per core
N = BPC * L                # windows per core
FT = 512                   # free-axis tile (windows per group)
NG = N // FT               # groups per core
PAD = KS - 1               # left pad columns in gi (per batch row!)
GPR = L // FT              # groups per batch row
GIW = BPC * (L + PAD)      # gi buffer width (pad head before each batch row)

_cache: dict = {}


def _build_module():
    nc = bacc.Bacc(
        "TRN2",
        target_bir_lowering=False,
        debug=False,
        num_devices=NCORES,
    )

    # ---- per-core DRAM I/O ----
    xT_d = nc.dram_tensor("xT", [128, 2, N], F16, kind="ExternalInput").ap()
    wih_d = nc.dram_tensor("wih", [128, 2, 768], F16, kind="ExternalInput").ap()
    whh_d = nc.dram_tensor("whh", [128, 2, 768], F16, kind="ExternalInput").ap()
    gibias_d = nc.dram_tensor("gibias", [128, 6], F32, kind="ExternalInput").ap()
    gipad_d = nc.dram_tensor("gipad", [128, 6, PAD], F16, kind="ExternalInput").ap()
    biasb_d = nc.dram_tensor("biasb", [1, 256], F16, kind="ExternalInput").ap()
    ident_d = nc.dram_tensor("ident", [128, 128], F16, kind="ExternalInput").ap()
    ones_d = nc.dram_tensor("ones", [1, FT], F16, kind="ExternalInput").ap()
    out_d = nc.dram_tensor("out16", [N, 256], F16, kind="ExternalOutput").ap()

    with TileContext(nc) as tc:
        with (
            tc.tile_pool(name="consts", bufs=1) as consts,
            tc.tile_pool(name="bigbuf", bufs=1) as bigbuf,
            tc.tile_pool(name="state", bufs=1) as state,
        ):
            wih = consts.tile([128, 2, 768], F16)
            whh = consts.tile([128, 2, 768], F16)
            gibias = consts.tile([128, 6], F32)
            biasb = consts.tile([1, 256], F16)
            ident = consts.tile([128, 128], F16)
            ones = consts.tile([1, FT], F16)
            xT = bigbuf.tile([128, 2, N], F16)
            gi = bigbuf.tile([128, 6, GIW], F16)

            nc.sync.dma_start(out=wih, in_=wih_d)
            nc.sync.dma_start(out=whh, in_=whh_d)
            nc.sync.dma_start(out=gibias, in_=gibias_d)
            nc.sync.dma_start(out=biasb, in_=biasb_d)
            nc.sync.dma_start(out=ident, in_=ident_d)
            nc.sync.dma_start(out=ones, in_=ones_d)
            nc.sync.dma_start(out=xT, in_=xT_d)
            for rb in range(BPC):
                base = rb * (L + PAD)
                nc.sync.dma_start(out=gi[:, :, base : base + PAD], in_=gipad_d)

            # ---- phase 1: gi precompute (once per position) ----
            with tc.tile_pool(name="gips", bufs=4, space="PSUM") as gips:
                for c in range(N // FT):
                    gio = c * FT + (c // GPR + 1) * PAD  # pad head per batch row
                    for s in range(6):
                        pt = gips.tile([128, FT], F32)
                        nc.tensor.matmul(
                            pt,
                            wih[:, 0, s * 128 : (s + 1) * 128],
                            xT[:, 0, c * FT : (c + 1) * FT],
                            start=True,
                            stop=False,
                        )
                        nc.tensor.matmul(
                            pt,
                            wih[:, 1, s * 128 : (s + 1) * 128],
                            xT[:, 1, c * FT : (c + 1) * FT],
                            start=False,
                            stop=True,
                        )
                        nc.vector.tensor_scalar(
                            out=gi[:, s, gio : gio + FT],
                            in0=pt,
                            scalar1=gibias[:, s : s + 1],
                            scalar2=None,
                            op0=ALU.add,
                        )

            # ---- phase 2: recurrence ----
            ht = []
            for g in range(NG):
                t = state.tile([128, 2, FT], F16, tag=f"ht{g}")
                nc.vector.memset(t, 0.5)
                ht.append(t)

            with (
                tc.tile_pool(name="psAr", bufs=1, space="PSUM") as psAr,
                tc.tile_pool(name="psAz", bufs=1, space="PSUM") as psAz,
                tc.tile_pool(name="psB", bufs=2, space="PSUM") as psB,
                tc.tile_pool(name="gates", bufs=2) as gates,
            ):
                for k in range(KS):
                    for g in range(NG):
                        o = g * FT + (g // GPR) * PAD + k  # gi col offset (pad-adj)
                        hg = ht[g]

                        # a_r / a_z into PSUM: W_hh' @ ht  (+ gi via identity MM)
                        Ar = psAr.tile([128, 2, FT], F32, tag="Ar")
                        for t in range(2):
                            w0 = whh[:, 0, t * 128 : (t + 1) * 128]
                            w1 = whh[:, 1, t * 128 : (t + 1) * 128]
                            nc.tensor.matmul(Ar[:, t], w0, hg[:, 0], start=True, stop=False)
                            nc.tensor.matmul(Ar[:, t], w1, hg[:, 1], start=False, stop=False)
                            nc.tensor.matmul(
                                Ar[:, t], ident, gi[:, t, o : o + FT], start=False, stop=True
                            )
                        r = gates.tile([128, 2, FT], F16, tag="r")
                        nc.scalar.activation(r, Ar, AF.Sigmoid)

                        Az = psAz.tile([128, 2, FT], F32, tag="Az")
                        for t in range(2):
                            w0 = whh[:, 0, (2 + t) * 128 : (3 + t) * 128]
                            w1 = whh[:, 1, (2 + t) * 128 : (3 + t) * 128]
                            nc.tensor.matmul(Az[:, t], w0, hg[:, 0], start=True, stop=False)
                            nc.tensor.matmul(Az[:, t], w1, hg[:, 1], start=False, stop=False)
                            nc.tensor.matmul(
                                Az[:, t], ident, gi[:, 2 + t, o : o + FT], start=False, stop=True
                            )
                        z = gates.tile([128, 2, FT], F16, tag="z")
                        nc.scalar.activation(z, Az, AF.Sigmoid)

                        # B = 4*W_hh_n @ ht + bias_B  (bias via K=1 ones outer product)
                        Bp = psB.tile([128, 2, FT], F32, tag="B")
                        for t in range(2):
                            nc.tensor.matmul(
                                Bp[:, t],
                                biasb[0:1, t * 128 : (t + 1) * 128],
                                ones,
                                start=True,
                                stop=False,
                            )
                            w0 = whh[:, 0, (4 + t) * 128 : (5 + t) * 128]
                            w1 = whh[:, 1, (4 + t) * 128 : (5 + t) * 128]
                            nc.tensor.matmul(Bp[:, t], w0, hg[:, 0], start=False, stop=False)
                            nc.tensor.matmul(Bp[:, t], w1, hg[:, 1], start=False, stop=True)

                        m = gates.tile([128, 2, FT], F16, tag="m")
                        nc.vector.tensor_mul(m, r, Bp)
                        an = gates.tile([128, 2, FT], F16, tag="an")
                        nc.vector.tensor_add(an, m, gi[:, 4:6, o : o + FT])
                        u = gates.tile([128, 2, FT], F16, tag="u")
                        nc.scalar.activation(u, an, AF.Sigmoid)

                        d = gates.tile([128, 2, FT], F16, tag="d")
                        nc.vector.tensor_sub(d, hg, u)
                        e = gates.tile([128, 2, FT], F16, tag="e")
                        nc.vector.tensor_mul(e, z, d)
                        nc.vector.tensor_add(hg, u, e)

            # ---- phase 3: transpose back + store ----
            with tc.tile_pool(name="stg", bufs=4) as stg:
                for g in range(NG):
                    for t in range(2):
                        s = stg.tile([128, FT // 128, 128], F16, tag="stg")
                        for ci in range(FT // 128):
                            nc.sync.dma_start_transpose(
                                out=s[:, ci, :],
                                in_=ht[g][:, t, ci * 128 : (ci + 1) * 128],
                            )
                        dst = out_d[
                            g * FT : (g + 1) * FT, t * 128 : (t + 1) * 128
                        ].rearrange("(di do) m -> do di m", do=128)
                        nc.sync.dma_start(out=dst, in_=s)

    nc.compile()
    return nc


def _prep_inputs(x, W_ih, W_hh, b_ih, b_hh):
    f16 = np.float16
    x = np.asarray(x, np.float32)
    W_ih = np.asarray(W_ih, np.float32)
    W_hh = np.asarray(W_hh, np.float32)
    b_ih = np.asarray(b_ih, np.float32)
    b_hh = np.asarray(b_hh, np.float32)

    onesD = np.ones(D, np.float32)
    # stationary weights (transposed to [K, M] layout, pre-scaled)
    wih_prep = np.concatenate([W_ih[:512], 2.0 * W_ih[512:]], axis=0).T  # (256,768)
    whh_prep = np.concatenate([2.0 * W_hh[:512], 4.0 * W_hh[512:]], axis=0).T
    wih_np = np.ascontiguousarray(
        wih_prep.reshape(2, 128, 768).transpose(1, 0, 2)
    ).astype(f16)
    whh_np = np.ascontiguousarray(
        whh_prep.reshape(2, 128, 768).transpose(1, 0, 2)
    ).astype(f16)

    bias_rz = b_ih[:512] + b_hh[:512] - W_hh[:512] @ onesD
    bias_gin = 2.0 * b_ih[512:]
    vec768 = np.concatenate([bias_rz, bias_gin])              # gi bias, folded
    gibias_np = np.ascontiguousarray(vec768.reshape(6, 128).T).astype(np.float32)
    gipad_np = np.ascontiguousarray(
        np.broadcast_to(f16(vec768).reshape(6, 128).T[:, :, None], (128, 6, PAD))
    ).astype(f16)
    biasb_np = (2.0 * (b_hh[512:] - W_hh[512:] @ onesD)).reshape(1, 256).astype(f16)
    ident_np = np.eye(128, dtype=f16)
    ones_np = np.ones((1, FT), f16)

    in_maps = []
    for c in range(NCORES):
        shard = x[c * BPC : (c + 1) * BPC].reshape(N, D)
        xT_np = np.ascontiguousarray(
            shard.T.reshape(2, 128, N).transpose(1, 0, 2)
        ).astype(f16)
        in_maps.append(
            {
                "xT": xT_np,
                "wih": wih_np,
                "whh": whh_np,
                "gibias": gibias_np,
                "gipad": gipad_np,
                "biasb": biasb_np,
                "ident": ident_np,
                "ones": ones_np,
            }
        )
    return in_maps


def run(x, W_ih, W_hh, b_ih, b_hh, trace=False, **run_kwargs):
    if "nc" not in _cache:
        _cache["nc"] = _build_module()
    nc = _cache["nc"]
    in_maps = _prep_inputs(x, W_ih, W_hh, b_ih, b_hh)
    res = bass_utils.run_bass_kernel_spmd(
        nc, in_maps, core_ids=list(range(NCORES)), trace=trace, **run_kwargs
    )
    outs = []
    for c in range(NCORES):
        ht16 = res.results[c]["out16"]          # (N, 256) fp16, sigmoid-space
        h = 2.0 * ht16.astype(np.float32) - 1.0
        outs.append(h.reshape(BPC, L, D))
    full = np.concatenate(outs, axis=0)
    return full, res


def kernel(x, W_ih, W_hh, b_ih, b_hh, ksize=KS):
    assert int(ksize) == KS
    full, _ = run(x, W_ih, W_hh, b_ih, b_hh, trace=False)
    return full


# revision 12
# speedup vs baseline: 1.0370x; 1.0370x over previous
"""Trainium2 Bass kernel for LocalRNN (sliding-window GRU, B=16 L=1024 D=256 K=16).

Strategy (8 NeuronCores, data-parallel over batch, 2 batch rows / core):
  - For each of the 2048 positions per core, a 16-step GRU runs over the
    trailing 16-position window (zero left-pad).  All math is done in a
    feature-major layout: features on SBUF partitions (256 = 2x128 halves),
    window index on the free axis.
  - gi = W_ih @ x_t is precomputed once per *position* (16x reuse across the
    overlapping windows) and stored fp16 in SBUF with a 15-column pad head.
  - The GRU is rewritten in sigmoid space:  h = 2*ht - 1, so that
        ht' = u + z*(ht - u),  u = sigmoid(2*a_n)
    and tanh disappears (tanh(v) = 2*sigmoid(2v) - 1).  All gate scales and
    biases are folded into the (host-prepared) weights:
        a_rz = gi_rz + (2*W_hh_rz) @ ht
        B    = (4*W_hh_n) @ ht + 2*(b_hh_n - W_hh_n@1)
        a_n  = gi_n + r * B
    gi_rz carries b_ih_rz + b_hh_rz - W_hh_rz@1 and gi_n carries 2*b_ih_n.
  - Matmuls accumulate in fp32 PSUM; gi adds for r/z ride the same PSUM
    accumulation group via identity matmuls, so the sigmoid reads PSUM
    directly.  Elementwise runs fp16 on the vector engine (2x mode).
  - Output ht is xbar-DMA-transposed back to row-major fp16; the final
    affine 2*ht-1 and fp32 cast happen on the host.
"""

import numpy as np

import concourse.bacc as bacc
import concourse.bass as bass
import concourse.mybir as mybir
from concourse import bass_utils
from concourse.tile import TileContext

F16 = mybir.dt.float16
F32 = mybir.dt.float32
AF = mybir.ActivationFunctionType
ALU = mybir.AluOpType

B, L, D, KS = 16, 1024, 256, 16
NCORES = 8
BPC = B // NCORES          # batch rows per core
N = BPC * L                # windows per core
FT = 512                   # free-axis tile (windows per group)
NG = N // FT               # groups per core
PAD = KS - 1               # left pad columns in gi (per batch row!)
GPR = L // FT              # groups per batch row
GIW = BPC * (L + PAD)      # gi buffer width (pad head before each batch row)

_cache: dict = {}


def _build_module():
    nc = bacc.Bacc(
        "TRN2",
        target_bir_lowering=False,
        debug=False,
        num_devices=NCORES,
    )

    # ---- per-core DRAM I/O ----
    xT_d = nc.dram_tensor("xT", [128, 2, N], F16, kind="ExternalInput").ap()
    wih_d = nc.dram_tensor("wih", [128, 2, 768], F16, kind="ExternalInput").ap()
    whh_d = nc.dram_tensor("whh", [128, 2, 768], F16, kind="ExternalInput").ap()
    gibias_d = nc.dram_tensor("gibias", [128, 6], F32, kind="ExternalInput").ap()
    gipad_d = nc.dram_tensor("gipad", [128, 6, PAD], F16, kind="ExternalInput").ap()
    biasb_d = nc.dram_tensor("biasb", [128, 2], F32, kind="ExternalInput").ap()
    ident_d = nc.dram_tensor("ident", [128, 128], F16, kind="ExternalInput").ap()
    out_d = nc.dram_tensor("out16", [N, 256], F16, kind="ExternalOutput").ap()

    with TileContext(nc) as tc:
        with (
            tc.tile_pool(name="consts", bufs=1) as consts,
            tc.tile_pool(name="bigbuf", bufs=1) as bigbuf,
            tc.tile_pool(name="state", bufs=1) as state,
        ):
            wih = consts.tile([128, 2, 768], F16)
            whh = consts.tile([128, 2, 768], F16)
            gibias = consts.tile([128, 6], F32)
            biasb = consts.tile([128, 2], F32)
            ident = consts.tile([128, 128], F16)
            xT = bigbuf.tile([128, 2, N], F16)
            gi = bigbuf.tile([128, 6, GIW], F16)

            nc.sync.dma_start(out=wih, in_=wih_d)
            nc.sync.dma_start(out=whh, in_=whh_d)
            nc.sync.dma_start(out=gibias, in_=gibias_d)
            nc.sync.dma_start(out=biasb, in_=biasb_d)
            nc.sync.dma_start(out=ident, in_=ident_d)
            nc.sync.dma_start(out=xT, in_=xT_d)
            for rb in range(BPC):
                base = rb * (L + PAD)
                nc.sync.dma_start(out=gi[:, :, base : base + PAD], in_=gipad_d)

            # ---- phase 1: gi precompute (once per position) ----
            with tc.tile_pool(name="gips", bufs=4, space="PSUM") as gips:
                for c in range(N // FT):
                    gio = c * FT + (c // GPR + 1) * PAD  # pad head per batch row
                    for s in range(6):
                        pt = gips.tile([128, FT], F32)
                        nc.tensor.matmul(
                            pt,
                            wih[:, 0, s * 128 : (s + 1) * 128],
                            xT[:, 0, c * FT : (c + 1) * FT],
                            start=True,
                            stop=False,
                        )
                        nc.tensor.matmul(
                            pt,
                            wih[:, 1, s * 128 : (s + 1) * 128],
                            xT[:, 1, c * FT : (c + 1) * FT],
                            start=False,
                            stop=True,
                        )
                        nc.vector.tensor_scalar(
                            out=gi[:, s, gio : gio + FT],
                            in0=pt,
                            scalar1=gibias[:, s : s + 1],
                            scalar2=None,
                            op0=ALU.add,
                        )

            # ---- phase 2: recurrence ----
            ht = []
            for g in range(NG):
                t = state.tile([128, 2, FT], F16, tag=f"ht{g}")
                nc.vector.memset(t, 0.5)
                ht.append(t)

            with (
                tc.tile_pool(name="psAr", bufs=1, space="PSUM") as psAr,
                tc.tile_pool(name="psAz", bufs=1, space="PSUM") as psAz,
                tc.tile_pool(name="psB", bufs=2, space="PSUM") as psB,
                tc.tile_pool(name="gates", bufs=2) as gates,
            ):
                for k in range(KS):
                    for g in range(NG):
                        o = g * FT + (g // GPR) * PAD + k  # gi col offset (pad-adj)
                        hg = ht[g]

                        # B_raw = 4*W_hh_n @ ht  (bias added in the stt mul below)
                        Bp = psB.tile([128, 2, FT], F32, tag="B")
                        for t in range(2):
                            w0 = whh[:, 0, (4 + t) * 128 : (5 + t) * 128]
                            w1 = whh[:, 1, (4 + t) * 128 : (5 + t) * 128]
                            nc.tensor.matmul(Bp[:, t], w0, hg[:, 0], start=True, stop=False)
                            nc.tensor.matmul(Bp[:, t], w1, hg[:, 1], start=False, stop=True)

                        # a_r / a_z into PSUM: W_hh' @ ht  (+ gi via identity MM)
                        Ar = psAr.tile([128, 2, FT], F32, tag="Ar")
                        for t in range(2):
                            w0 = whh[:, 0, t * 128 : (t + 1) * 128]
                            w1 = whh[:, 1, t * 128 : (t + 1) * 128]
                            nc.tensor.matmul(Ar[:, t], w0, hg[:, 0], start=True, stop=False)
                            nc.tensor.matmul(Ar[:, t], w1, hg[:, 1], start=False, stop=False)
                            nc.tensor.matmul(
                                Ar[:, t], ident, gi[:, t, o : o + FT], start=False, stop=True
                            )
                        r = gates.tile([128, 2, FT], F16, tag="r")
                        nc.scalar.activation(r, Ar, AF.Sigmoid)

                        Az = psAz.tile([128, 2, FT], F32, tag="Az")
                        for t in range(2):
                            w0 = whh[:, 0, (2 + t) * 128 : (3 + t) * 128]
                            w1 = whh[:, 1, (2 + t) * 128 : (3 + t) * 128]
                            nc.tensor.matmul(Az[:, t], w0, hg[:, 0], start=True, stop=False)
                            nc.tensor.matmul(Az[:, t], w1, hg[:, 1], start=False, stop=False)
                            nc.tensor.matmul(
                                Az[:, t], ident, gi[:, 2 + t, o : o + FT], start=False, stop=True
                            )
                        z = gates.tile([128, 2, FT], F16, tag="z")
                        nc.scalar.activation(z, Az, AF.Sigmoid)

                        # m = (B_raw + bias_B) * r   via fused scalar_tensor_tensor
                        m = gates.tile([128, 2, FT], F16, tag="m")
                        for t in range(2):
                            nc.vector.scalar_tensor_tensor(
                                out=m[:, t],
                                in0=Bp[:, t],
                                scalar=biasb[:, t : t + 1],
                                in1=r[:, t],
                                op0=ALU.add,
                                op1=ALU.mult,
                            )
                        an = gates.tile([128, 2, FT], F16, tag="an")
                        nc.vector.tensor_add(an, m, gi[:, 4:6, o : o + FT])
                        u = gates.tile([128, 2, FT], F16, tag="u")
                        nc.scalar.activation(u, an, AF.Sigmoid)

                        d = gates.tile([128, 2, FT], F16, tag="d")
                        nc.vector.tensor_sub(d, hg, u)
                        e = gates.tile([128, 2, FT], F16, tag="e")
                        nc.vector.tensor_mul(e, z, d)
                        nc.vector.tensor_add(hg, u, e)

            # ---- phase 3: transpose back + store (split across both HWDGE engines) ----
            with tc.tile_pool(name="stg", bufs=4) as stg:
                for g in range(NG):
                    for t in range(2):
                        eng = nc.sync if (g * 2 + t) % 2 == 0 else nc.scalar
                        s = stg.tile([128, FT // 128, 128], F16, tag="stg")
                        eng.dma_start_transpose(out=s, in_=ht[g][:, t, :])
                        dst = out_d[
                            g * FT : (g + 1) * FT, t * 128 : (t + 1) * 128
                        ].rearrange("(di do) m -> do di m", do=128)
                        eng.dma_start(out=dst, in_=s)

    nc.compile()
    return nc


def _prep_inputs(x, W_ih, W_hh, b_ih, b_hh):
    f16 = np.float16
    x = np.asarray(x, np.float32)
    W_ih = np.asarray(W_ih, np.float32)
    W_hh = np.asarray(W_hh, np.float32)
    b_ih = np.asarray(b_ih, np.float32)
    b_hh = np.asarray(b_hh, np.float32)

    onesD = np.ones(D, np.float32)
    # stationary weights (transposed to [K, M] layout, pre-scaled)
    wih_prep = np.concatenate([W_ih[:512], 2.0 * W_ih[512:]], axis=0).T  # (256,768)
    whh_prep = np.concatenate([2.0 * W_hh[:512], 4.0 * W_hh[512:]], axis=0).T
    wih_np = np.ascontiguousarray(
        wih_prep.reshape(2, 128, 768).transpose(1, 0, 2)
    ).astype(f16)
    whh_np = np.ascontiguousarray(
        whh_prep.reshape(2, 128, 768).transpose(1, 0, 2)
    ).astype(f16)

    bias_rz = b_ih[:512] + b_hh[:512] - W_hh[:512] @ onesD
    bias_gin = 2.0 * b_ih[512:]
    vec768 = np.concatenate([bias_rz, bias_gin])              # gi bias, folded
    gibias_np = np.ascontiguousarray(vec768.reshape(6, 128).T).astype(np.float32)
    gipad_np = np.ascontiguousarray(
        np.broadcast_to(f16(vec768).reshape(6, 128).T[:, :, None], (128, 6, PAD))
    ).astype(f16)
    biasb_np = np.ascontiguousarray(
        (2.0 * (b_hh[512:] - W_hh[512:] @ onesD)).reshape(2, 128).T
    ).astype(np.float32)
    ident_np = np.eye(128, dtype=f16)

    in_maps = []
    for c in range(NCORES):
        shard = x[c * BPC : (c + 1) * BPC].reshape(N, D)
        xT_np = np.ascontiguousarray(
            shard.T.reshape(2, 128, N).transpose(1, 0, 2)
        ).astype(f16)
        in_maps.append(
            {
                "xT": xT_np,
                "wih": wih_np,
                "whh": whh_np,
                "gibias": gibias_np,
                "gipad": gipad_np,
                "biasb": biasb_np,
                "ident": ident_np,
            }
        )
    return in_maps


def run(x, W_ih, W_hh, b_ih, b_hh, trace=False, **run_kwargs):
    if "nc" not in _cache:
        _cache["nc"] = _build_module()
    nc = _cache["nc"]
    in_maps = _prep_inputs(x, W_ih, W_hh, b_ih, b_hh)
    res = bass_utils.run_bass_kernel_spmd(
        nc, in_maps, core_ids=list(range(NCORES)), trace=trace, **run_kwargs
    )
    outs = []
    for c in range(NCORES):
        ht16 = res.results[c]["out16"]          # (N, 256) fp16, sigmoid-space
        h = 2.0 * ht16.astype(np.float32) - 1.0
        outs.append(h.reshape(BPC, L, D))
    full = np.concatenate(outs, axis=0)
    return full, res


def kernel(x, W_ih, W_hh, b_ih, b_hh, ksize=KS):
    assert int(ksize) == KS
    full, _ = run(x, W_ih, W_hh, b_ih, b_hh, trace=False)
    return full
